# revision 9
# baseline (speedup 1.0000x reference)
"""Trainium2 Bass kernel for NeuronGCN (GCN -> SAGE -> GAT -> pool -> MLP head).

Sharding: nodes partitioned across 8 cores (12500 each), edges partitioned by
destination node. Within each core, owned nodes are sorted by in-degree
(descending) and packed into 128-row blocks; each node's in-edges occupy "slot"
columns of its block (slot k of block b lives at a shared column index), padded
with a pointer to a known all-zero table row. Per-edge work becomes:
one multi-row indirect DMA gather per block + free-axis reductions. Weights are
replicated; BN statistics and pooled per-graph sums are AllReduced; per-node
feature tables (gather sources) are AllGathered between layers.

All floating-point math runs on device. The host only does integer graph
layout (sharding, degree-sort permutation, slot/column assignment, dtype
casts) and data movement.
"""

import sys

sys.path.insert(0, "/opt/trn_rl_repo")

import numpy as np

import concourse.bass as bass
import concourse.bacc as bacc
import concourse.tile as tile
from concourse import mybir
from concourse.masks import make_identity
from concourse.bass_utils import run_bass_kernel_spmd

F32 = mybir.dt.float32
I32 = mybir.dt.int32

# problem constants
N = 100000
E = 800000
IN = 128
H = 64
HEADS = 2
B = 64
OUT = 10
EPS = 1e-5
NEG = 0.2
NCORES = 8
P = 128


# --------------------------------------------------------------------------
# host-side graph layout (integer/index work only)
# --------------------------------------------------------------------------

def host_prep(x, edge_index, batch, ncores):
    n = x.shape[0]
    npc = n // ncores
    assert npc * ncores == n and npc % P != 0
    npad = ((npc + P - 1) // P) * P
    nblk = npad // P
    vlast = npc - P * (nblk - 1)

    src = np.asarray(edge_index[0], dtype=np.int64)
    dst = np.asarray(edge_index[1], dtype=np.int64)
    batch = np.asarray(batch, dtype=np.int64)
    indeg = np.bincount(dst, minlength=n)

    # per-core node permutation: owned nodes sorted by in-degree descending
    gid = np.empty(n, dtype=np.int64)          # orig node -> global permuted row
    perms = []
    for c in range(ncores):
        vs = np.arange(c * npc, (c + 1) * npc)
        order = np.argsort(-indeg[vs], kind="stable")
        pc = vs[order]
        perms.append(pc)
        gid[pc] = c * npad + np.arange(npc)

    # shared slot structure: Kb[b] = max in-degree within block b across cores
    kb = np.zeros(nblk, dtype=np.int64)
    for c in range(ncores):
        d = indeg[perms[c]]
        d = np.concatenate([d, np.zeros(npad - npc, np.int64)])
        kb = np.maximum(kb, d.reshape(nblk, P).max(axis=1))
    colstart = np.zeros(nblk + 1, dtype=np.int64)
    colstart[1:] = np.cumsum(kb + 1)           # +1 self slot per block
    s_total = int(colstart[-1])
    zrow = npad - 1                            # core-0 pad row: all-zero in every table

    owner = dst // npc
    idx_all = np.full((ncores, P, s_total), zrow, dtype=np.int32)
    for c in range(ncores):
        m = owner == c
        es, ed = src[m], dst[m]
        lpos = gid[ed] - c * npad              # local slot position [0, npc)
        o = np.argsort(lpos, kind="stable")
        lpos, es = lpos[o], es[o]
        counts = np.bincount(lpos, minlength=npc)
        starts = np.concatenate([[0], np.cumsum(counts)[:-1]])
        k = np.arange(len(lpos)) - starts[lpos]
        bb, pp = lpos // P, lpos % P
        idx_all[c, pp, colstart[bb] + k] = gid[es].astype(np.int32)
        # self slot (last column of each block's range) for real nodes
        lp = np.arange(npc)
        idx_all[c, lp % P, colstart[lp // P] + kb[lp // P]] = (
            c * npad + lp
        ).astype(np.int32)

    xt_all = np.zeros((ncores, x.shape[1], npad), dtype=np.float32)
    bcols_all = np.full((ncores, P, nblk), float(B), dtype=np.float32)
    for c in range(ncores):
        xt_all[c, :, :npc] = x[perms[c]].T
        bc = np.full(npad, B, np.int64)
        bc[:npc] = batch[perms[c]]
        bcols_all[c] = bc.reshape(nblk, P).T.astype(np.float32)

    cfg = dict(
        ncores=ncores, npc=npc, npad=npad, nblk=nblk, vlast=vlast,
        kb=[int(v) for v in kb], colstart=[int(v) for v in colstart],
        s_total=s_total, zrow=zrow, ntot=n, in_dim=x.shape[1],
    )
    return cfg, xt_all, idx_all, bcols_all


# --------------------------------------------------------------------------
# device program
# --------------------------------------------------------------------------

def build_program(cfg, dbg=False):
    ncores = cfg["ncores"]
    npad, nblk, vlast = cfg["npad"], cfg["nblk"], cfg["vlast"]
    kb, colstart, s_total = cfg["kb"], cfg["colstart"], cfg["s_total"]
    zrow, ntot, in_dim = cfg["zrow"], cfg["ntot"], cfg["in_dim"]
    kmax = max(kb) + 1
    grp = [list(range(ncores))]

    nc = bacc.Bacc("TRN2", target_bir_lowering=False, debug=False,
                   num_devices=ncores)

    # I/O
    xT_d = nc.dram_tensor("xT", [in_dim, npad], F32, kind="ExternalInput")
    idx_d = nc.dram_tensor("idx", [P, s_total], I32, kind="ExternalInput")
    bcols_d = nc.dram_tensor("bcols", [P, nblk], F32, kind="ExternalInput")
    wnames = {
        "W_gcn": [in_dim, H], "W_sl": [H, H], "W_sr": [H, H],
        "W_gat": [H, HEADS * H], "W_m1": [2 * H, H], "W_m2": [H, OUT],
        "b_gcn": [1, H], "b_sage": [1, H], "b_gat": [1, H],
        "att_s": [1, HEADS * H], "att_d": [1, HEADS * H],
        "g1": [1, H], "be1": [1, H], "g2": [1, H], "be2": [1, H],
        "g3": [1, H], "be3": [1, H], "gm": [1, H], "bem": [1, H],
        "b_m1": [1, H], "b_m2": [1, OUT],
    }
    w_d = {k: nc.dram_tensor(k, shp, F32, kind="ExternalInput")
           for k, shp in wnames.items()}
    out_d = nc.dram_tensor("out", [B, OUT], F32, kind="ExternalOutput")
    GWc = HEADS * H + HEADS
    if dbg:
        dbg_d = {
            "dbg_deg": nc.dram_tensor("dbg_deg", [P, nblk], F32, kind="ExternalOutput"),
            "dbg_dinv": nc.dram_tensor("dbg_dinv", [P, nblk], F32, kind="ExternalOutput"),
            "dbg_x1": nc.dram_tensor("dbg_x1", [P, nblk * H], F32, kind="ExternalOutput"),
            "dbg_st1": nc.dram_tensor("dbg_st1", [1, 2 * H], F32, kind="ExternalOutput"),
            "dbg_x1n": nc.dram_tensor("dbg_x1n", [npad, H], F32, kind="ExternalOutput"),
            "dbg_x2": nc.dram_tensor("dbg_x2", [P, nblk * H], F32, kind="ExternalOutput"),
            "dbg_xg": nc.dram_tensor("dbg_xg", [npad, GWc], F32, kind="ExternalOutput"),
            "dbg_ad": nc.dram_tensor("dbg_ad", [P, nblk * HEADS], F32, kind="ExternalOutput"),
            "dbg_x3": nc.dram_tensor("dbg_x3", [P, nblk * H], F32, kind="ExternalOutput"),
            "dbg_pool": nc.dram_tensor("dbg_pool", [B, H + 1], F32, kind="ExternalOutput"),
        }

    from contextlib import ExitStack
    with tile.TileContext(nc) as tc, ExitStack() as stk:
        cst = stk.enter_context(tc.tile_pool(name="cst", bufs=1))
        wk = stk.enter_context(tc.tile_pool(name="wk", bufs=2))
        gp = stk.enter_context(tc.tile_pool(name="gp", bufs=2))
        ps_mm = stk.enter_context(tc.tile_pool(name="ps_mm", bufs=2, space="PSUM"))
        ps_tp = stk.enter_context(tc.tile_pool(name="ps_tp", bufs=2, space="PSUM"))
        ps_st = stk.enter_context(tc.tile_pool(name="ps_st", bufs=2, space="PSUM"))
        dr = stk.enter_context(tc.tile_pool(name="dr", bufs=1, space="DRAM"))

        # ---- constants ----
        ident = cst.tile([P, P], F32, name="ident")
        make_identity(nc, ident[:])
        ones_row = cst.tile([1, P], F32, name="ones_row")
        nc.vector.memset(ones_row[:], 1.0)
        ones_col = cst.tile([P, 1], F32, name="ones_col")
        nc.vector.memset(ones_col[:], 1.0)
        eps_sb = cst.tile([1, 1], F32, name="eps_sb")
        nc.vector.memset(eps_sb[:], EPS)
        iota_i = cst.tile([P, B], I32, name="iota_i")
        nc.gpsimd.iota(iota_i[:], pattern=[[1, B]], base=0, channel_multiplier=0)
        iota_f = cst.tile([P, B], F32, name="iota_f")
        nc.vector.tensor_copy(out=iota_f[:], in_=iota_i[:])
        pidx_i = cst.tile([P, 1], I32, name="pidx_i")
        nc.gpsimd.iota(pidx_i[:], pattern=[[1, 1]], base=0, channel_multiplier=1)
        pidx_f = cst.tile([P, 1], F32, name="pidx_f")
        nc.vector.tensor_copy(out=pidx_f[:], in_=pidx_i[:])
        vmask = cst.tile([P, 1], F32, name="vmask")
        nc.vector.tensor_scalar(out=vmask[:], in0=pidx_f[:],
                                scalar1=float(vlast), scalar2=None,
                                op0=mybir.AluOpType.is_lt)

        w_sb = {}
        for k, shp in wnames.items():
            w_sb[k] = cst.tile(shp, F32, name=f"w_{k}")
            nc.sync.dma_start(out=w_sb[k][:], in_=w_d[k][:])

        idx_sb = cst.tile([P, s_total], I32, name="idx_sb")
        nc.sync.dma_start(out=idx_sb[:], in_=idx_d[:])
        bcols_sb = cst.tile([P, nblk], F32, name="bcols_sb")
        nc.sync.dma_start(out=bcols_sb[:], in_=bcols_d[:])

        def bcast_row(row_ap, width, name):
            """materialize [P, width] tile with every row = row_ap ([1, width])"""
            t_ps = ps_mm.tile([P, width], F32, tag="mm", name=f"bc_{name}_ps")
            nc.tensor.matmul(out=t_ps[:], lhsT=ones_row[:], rhs=row_ap,
                             start=True, stop=True)
            t = cst.tile([P, width], F32, name=f"bc_{name}")
            nc.vector.tensor_copy(out=t[:], in_=t_ps[:])
            return t

        bgcn_t = bcast_row(w_sb["b_gcn"][:], H, "bgcn")
        bsage_t = bcast_row(w_sb["b_sage"][:], H, "bsage")
        bgat_t = bcast_row(w_sb["b_gat"][:], H, "bgat")
        atts_t = bcast_row(w_sb["att_s"][:], HEADS * H, "atts")
        attd_t = bcast_row(w_sb["att_d"][:], HEADS * H, "attd")

        # ---- residents ----
        x1_own = cst.tile([P, nblk * H], F32, name="x1_own")
        x2_own = cst.tile([P, nblk * H], F32, name="x2_own")
        x3_own = cst.tile([P, nblk * H], F32, name="x3_own")
        ad_own = cst.tile([P, nblk * HEADS], F32, name="ad_own")
        dinv_sb = cst.tile([P, nblk], F32, name="dinv_sb")
        cntinv_sb = cst.tile([P, nblk], F32, name="cntinv_sb")

        # DRAM tables + bounce buffers
        xws_c = dr.tile([npad, H], F32, name="xws_c")
        x1n_c = dr.tile([npad, H], F32, name="x1n_c")
        xg_c = dr.tile([npad, HEADS * H + HEADS], F32, name="xg_c")
        xws_full = dr.tile([ncores * npad, H], F32, name="xws_full",
                           addr_space="Shared")
        x1n_full = dr.tile([ncores * npad, H], F32, name="x1n_full",
                           addr_space="Shared")
        xg_full = dr.tile([ncores * npad, HEADS * H + HEADS], F32,
                          name="xg_full", addr_space="Shared")
        GW = HEADS * H + HEADS  # packed gat row width: xg(128) | a_s(2)

        def stats_allreduce(acc_sb, name):
            """AllReduce a [1, 2H] stats row; returns sbuf [1, 2H]."""
            st_c = dr.tile([1, 2 * H], F32, name=f"st_{name}_c")
            st_f = dr.tile([1, 2 * H], F32, name=f"st_{name}_f",
                           addr_space="Shared")
            nc.sync.dma_start(out=st_c[:], in_=acc_sb[:])
            nc.gpsimd.collective_compute(
                "AllReduce", mybir.AluOpType.add, replica_groups=grp,
                ins=[st_c[:]], outs=[st_f[:]])
            st_sb = cst.tile([1, 2 * H], F32, name=f"st_{name}_sb")
            nc.sync.dma_start(out=st_sb[:], in_=st_f[:])
            return st_sb

        def bn_coeffs(st_sb, g_row, be_row, count, name):
            """From [1,2H] (sum|sumsq) rows: A = g*rstd, C = be - mu*A,
            materialized as [P, H] tiles."""
            mu = cst.tile([1, H], F32, name=f"mu_{name}")
            nc.vector.tensor_scalar(out=mu[:], in0=st_sb[:, 0:H],
                                    scalar1=1.0 / count, scalar2=None,
                                    op0=mybir.AluOpType.mult)
            ex2 = cst.tile([1, H], F32, name=f"ex2_{name}")
            nc.vector.tensor_scalar(out=ex2[:], in0=st_sb[:, H:2 * H],
                                    scalar1=1.0 / count, scalar2=None,
                                    op0=mybir.AluOpType.mult)
            musq = cst.tile([1, H], F32, name=f"musq_{name}")
            nc.vector.tensor_tensor(out=musq[:], in0=mu[:], in1=mu[:],
                                    op=mybir.AluOpType.mult)
            var = cst.tile([1, H], F32, name=f"var_{name}")
            nc.vector.tensor_tensor(out=var[:], in0=ex2[:], in1=musq[:],
                                    op=mybir.AluOpType.subtract)
            std = cst.tile([1, H], F32, name=f"std_{name}")
            nc.scalar.activation(out=std[:], in_=var[:],
                                 func=mybir.ActivationFunctionType.Sqrt,
                                 bias=eps_sb[0:1, 0:1], scale=1.0)
            rstd = cst.tile([1, H], F32, name=f"rstd_{name}")
            nc.vector.reciprocal(out=rstd[:], in_=std[:])
            a_row = cst.tile([1, H], F32, name=f"arow_{name}")
            nc.vector.tensor_tensor(out=a_row[:], in0=g_row, in1=rstd[:],
                                    op=mybir.AluOpType.mult)
            mua = cst.tile([1, H], F32, name=f"mua_{name}")
            nc.vector.tensor_tensor(out=mua[:], in0=mu[:], in1=a_row[:],
                                    op=mybir.AluOpType.mult)
            c_row = cst.tile([1, H], F32, name=f"crow_{name}")
            nc.vector.tensor_tensor(out=c_row[:], in0=be_row, in1=mua[:],
                                    op=mybir.AluOpType.subtract)
            return bcast_row(a_row[:], H, f"A_{name}"), \
                bcast_row(c_row[:], H, f"C_{name}")

        def block_stats(x_ap, sq_src, acc_sb, b):
            """acc_sb[0,0:H] += col-sums of x, acc_sb[0,H:2H] += col-sums of x^2
            over valid rows of block b."""
            v = vlast if b == nblk - 1 else P
            sq = wk.tile([P, H], F32, name="sq_st")
            nc.scalar.activation(out=sq[:v], in_=sq_src[:v],
                                 func=mybir.ActivationFunctionType.Square)
            st_ps = ps_st.tile([1, 2 * H], F32, tag="st", name="st_ps")
            nc.tensor.matmul(out=st_ps[:, 0:H], lhsT=ones_col[:v],
                             rhs=x_ap[:v], start=True, stop=True)
            nc.tensor.matmul(out=st_ps[:, H:2 * H], lhsT=ones_col[:v],
                             rhs=sq[:v], start=True, stop=True)
            nc.vector.tensor_tensor(out=acc_sb[:], in0=acc_sb[:],
                                    in1=st_ps[:], op=mybir.AluOpType.add)

        def transpose_sb(src_ap, pdim, fdim, name):
            """PE transpose [pdim, fdim] -> sbuf [fdim, pdim]."""
            t_ps = ps_tp.tile([fdim, pdim], F32, tag="tp", name=f"{name}_ps")
            nc.tensor.transpose(out=t_ps[:], in_=src_ap,
                                identity=ident[:pdim, :pdim])
            t_sb = wk.tile([fdim, pdim], F32, name=f"{name}_sb", tag=name)
            nc.vector.tensor_copy(out=t_sb[:], in_=t_ps[:])
            return t_sb

        # ================= phase A: slot validity + degrees =================
        svalid = cst.tile([P, s_total], F32, name="svalid")
        nc.vector.tensor_copy(out=svalid[:], in_=idx_sb[:])
        nc.vector.tensor_scalar(out=svalid[:], in0=svalid[:],
                                scalar1=float(zrow), scalar2=None,
                                op0=mybir.AluOpType.not_equal)
        for b in range(nblk):
            k = kb[b]
            if k == 0:
                nc.vector.memset(dinv_sb[:, b:b + 1], 0.0)  # deg=0 marker
                continue
            c0 = colstart[b]
            nc.vector.reduce_sum(out=dinv_sb[:, b:b + 1],
                                 in_=svalid[:, c0:c0 + k],
                                 axis=mybir.AxisListType.X)
        # dinv_sb currently holds deg; derive cntinv = 1/max(deg,1), dinv = 1/sqrt(deg+1)
        if dbg:
            nc.sync.dma_start(out=dbg_d["dbg_deg"][:], in_=dinv_sb[:])
        tmp_deg = cst.tile([P, nblk], F32, name="tmp_deg")
        nc.vector.tensor_scalar(out=tmp_deg[:], in0=dinv_sb[:], scalar1=1.0,
                                scalar2=None, op0=mybir.AluOpType.max)
        nc.vector.reciprocal(out=cntinv_sb[:], in_=tmp_deg[:])
        sq_deg = cst.tile([P, nblk], F32, name="sq_deg")
        nc.scalar.activation(out=sq_deg[:], in_=dinv_sb[:],
                             func=mybir.ActivationFunctionType.Sqrt,
                             bias=1.0, scale=1.0)
        nc.vector.reciprocal(out=dinv_sb[:], in_=sq_deg[:])
        if dbg:
            nc.sync.dma_start(out=dbg_d["dbg_dinv"][:], in_=dinv_sb[:])

        # ================= phase B: xws table =================
        for b in range(nblk):
            xtb = wk.tile([in_dim, P], F32, name="xtb")
            nc.sync.dma_start(out=xtb[:], in_=xT_d[:, b * P:(b + 1) * P])
            xw_ps = ps_mm.tile([P, H], F32, tag="mm", name="xw_ps")
            nc.tensor.matmul(out=xw_ps[:], lhsT=xtb[:], rhs=w_sb["W_gcn"][:],
                             start=True, stop=True)
            xws = wk.tile([P, H], F32, name="xws")
            nc.vector.tensor_scalar(out=xws[:], in0=xw_ps[:],
                                    scalar1=dinv_sb[:, b:b + 1], scalar2=None,
                                    op0=mybir.AluOpType.mult)
            nc.sync.dma_start(out=xws_c[b * P:(b + 1) * P, :], in_=xws[:])

        nc.gpsimd.collective_compute(
            "AllGather", mybir.AluOpType.bypass, replica_groups=grp,
            ins=[xws_c[:]], outs=[xws_full[:]])

        # ================= phase D: GCN aggregate =================
        st1_acc = cst.tile([1, 2 * H], F32, name="st1_acc")
        nc.vector.memset(st1_acc[:], 0.0)
        for b in range(nblk):
            kg = kb[b] + 1
            c0 = colstart[b]
            g = gp.tile([P, kmax, H], F32, name="g_gcn", tag="g_gcn")
            for k in range(kg):
                nc.gpsimd.indirect_dma_start(
                    out=g[:, k, :], out_offset=None, in_=xws_full[:],
                    in_offset=bass.IndirectOffsetOnAxis(
                        ap=idx_sb[:, c0 + k:c0 + k + 1], axis=0))
            agg = wk.tile([P, H], F32, name="agg_gcn")
            nc.vector.reduce_sum(
                out=agg[:], in_=g[:, :kg, :].rearrange("p k f -> p f k"),
                axis=mybir.AxisListType.X)
            x1c = x1_own[:, b * H:(b + 1) * H]
            nc.vector.tensor_scalar(out=agg[:], in0=agg[:],
                                    scalar1=dinv_sb[:, b:b + 1], scalar2=None,
                                    op0=mybir.AluOpType.mult)
            nc.vector.tensor_tensor(out=x1c, in0=agg[:], in1=bgcn_t[:],
                                    op=mybir.AluOpType.add)
            block_stats(x1c, x1c, st1_acc, b)

        # BN1 -> x1n (in place) -> table
        if dbg:
            nc.sync.dma_start(out=dbg_d["dbg_x1"][:], in_=x1_own[:])
        st1 = stats_allreduce(st1_acc, "1")
        if dbg:
            nc.sync.dma_start(out=dbg_d["dbg_st1"][:], in_=st1[:])
        a1_t, c1_t = bn_coeffs(st1, w_sb["g1"][:], w_sb["be1"][:], ntot, "1")
        for b in range(nblk):
            x1c = x1_own[:, b * H:(b + 1) * H]
            nc.vector.tensor_tensor(out=x1c, in0=x1c, in1=a1_t[:],
                                    op=mybir.AluOpType.mult)
            nc.vector.tensor_tensor(out=x1c, in0=x1c, in1=c1_t[:],
                                    op=mybir.AluOpType.add)
            nc.vector.tensor_scalar(out=x1c, in0=x1c, scalar1=0.0,
                                    scalar2=None, op0=mybir.AluOpType.max)
            if b == nblk - 1:
                nc.vector.tensor_scalar(out=x1c, in0=x1c, scalar1=vmask[:],
                                        scalar2=None, op0=mybir.AluOpType.mult)
            nc.sync.dma_start(out=x1n_c[b * P:(b + 1) * P, :], in_=x1c)

        nc.gpsimd.collective_compute(
            "AllGather", mybir.AluOpType.bypass, replica_groups=grp,
            ins=[x1n_c[:]], outs=[x1n_full[:]])

        # ================= phase H: SAGE =================
        st2_acc = cst.tile([1, 2 * H], F32, name="st2_acc")
        nc.vector.memset(st2_acc[:], 0.0)
        for b in range(nblk):
            k = kb[b]
            c0 = colstart[b]
            agg = wk.tile([P, H], F32, name="agg_sage")
            if k == 0:
                nc.vector.memset(agg[:], 0.0)
            else:
                g = gp.tile([P, kmax, H], F32, name="g_sage", tag="g_gcn")
                for kk in range(k):
                    nc.gpsimd.indirect_dma_start(
                        out=g[:, kk, :], out_offset=None, in_=x1n_full[:],
                        in_offset=bass.IndirectOffsetOnAxis(
                            ap=idx_sb[:, c0 + kk:c0 + kk + 1], axis=0))
                nc.vector.reduce_sum(
                    out=agg[:], in_=g[:, :k, :].rearrange("p k f -> p f k"),
                    axis=mybir.AxisListType.X)
            nc.vector.tensor_scalar(out=agg[:], in0=agg[:],
                                    scalar1=cntinv_sb[:, b:b + 1],
                                    scalar2=None, op0=mybir.AluOpType.mult)
            aggT = transpose_sb(agg[:], P, H, "aggT")
            x1T = transpose_sb(x1_own[:, b * H:(b + 1) * H], P, H, "x1T")
            x2_ps = ps_mm.tile([P, H], F32, tag="mm", name="x2_ps")
            nc.tensor.matmul(out=x2_ps[:], lhsT=aggT[:], rhs=w_sb["W_sl"][:],
                             start=True, stop=False)
            nc.tensor.matmul(out=x2_ps[:], lhsT=x1T[:], rhs=w_sb["W_sr"][:],
                             start=False, stop=True)
            x2c = x2_own[:, b * H:(b + 1) * H]
            nc.vector.tensor_tensor(out=x2c, in0=x2_ps[:], in1=bsage_t[:],
                                    op=mybir.AluOpType.add)
            block_stats(x2c, x2c, st2_acc, b)

        if dbg:
            nc.sync.dma_start(out=dbg_d["dbg_x1n"][:], in_=x1n_c[:])
            nc.sync.dma_start(out=dbg_d["dbg_x2"][:], in_=x2_own[:])
        st2 = stats_allreduce(st2_acc, "2")
        a2_t, c2_t = bn_coeffs(st2, w_sb["g2"][:], w_sb["be2"][:], ntot, "2")

        # ================= phase J: BN2 + GAT table =================
        for b in range(nblk):
            x2c = x2_own[:, b * H:(b + 1) * H]
            nc.vector.tensor_tensor(out=x2c, in0=x2c, in1=a2_t[:],
                                    op=mybir.AluOpType.mult)
            nc.vector.tensor_tensor(out=x2c, in0=x2c, in1=c2_t[:],
                                    op=mybir.AluOpType.add)
            nc.vector.tensor_scalar(out=x2c, in0=x2c, scalar1=0.0,
                                    scalar2=None, op0=mybir.AluOpType.max)
            if b == nblk - 1:
                nc.vector.tensor_scalar(out=x2c, in0=x2c, scalar1=vmask[:],
                                        scalar2=None, op0=mybir.AluOpType.mult)
            x2T = transpose_sb(x2c, P, H, "x2T")
            xg_ps = ps_mm.tile([P, HEADS * H], F32, tag="mm", name="xg_ps")
            nc.tensor.matmul(out=xg_ps[:], lhsT=x2T[:], rhs=w_sb["W_gat"][:],
                             start=True, stop=True)
            pk = wk.tile([P, GW], F32, name="pk")
            nc.vector.tensor_copy(out=pk[:, 0:HEADS * H], in_=xg_ps[:])
            ta = wk.tile([P, HEADS * H], F32, name="ta")
            nc.vector.tensor_tensor(out=ta[:], in0=xg_ps[:], in1=atts_t[:],
                                    op=mybir.AluOpType.mult)
            for h in range(HEADS):
                nc.vector.reduce_sum(
                    out=pk[:, HEADS * H + h:HEADS * H + h + 1],
                    in_=ta[:, h * H:(h + 1) * H], axis=mybir.AxisListType.X)
            td = wk.tile([P, HEADS * H], F32, name="td")
            nc.vector.tensor_tensor(out=td[:], in0=xg_ps[:], in1=attd_t[:],
                                    op=mybir.AluOpType.mult)
            for h in range(HEADS):
                nc.vector.reduce_sum(
                    out=ad_own[:, HEADS * b + h:HEADS * b + h + 1],
                    in_=td[:, h * H:(h + 1) * H], axis=mybir.AxisListType.X)
            nc.sync.dma_start(out=xg_c[b * P:(b + 1) * P, :], in_=pk[:])

        nc.gpsimd.collective_compute(
            "AllGather", mybir.AluOpType.bypass, replica_groups=grp,
            ins=[xg_c[:]], outs=[xg_full[:]])

        # ================= phase L: GAT aggregate =================
        st3_acc = cst.tile([1, 2 * H], F32, name="st3_acc")
        nc.vector.memset(st3_acc[:], 0.0)
        for b in range(nblk):
            kg = kb[b] + 1
            c0 = colstart[b]
            g = gp.tile([P, kmax, GW], F32, name="g_gat", tag="g_gat")
            for k in range(kg):
                nc.gpsimd.indirect_dma_start(
                    out=g[:, k, :], out_offset=None, in_=xg_full[:],
                    in_offset=bass.IndirectOffsetOnAxis(
                        ap=idx_sb[:, c0 + k:c0 + k + 1], axis=0))
            hx = []
            for h in range(HEADS):
                eh = wk.tile([P, kmax], F32, name="eh", tag="eh")
                nc.vector.tensor_scalar(
                    out=eh[:, :kg],
                    in0=g[:, :kg, HEADS * H + h:HEADS * H + h + 1],
                    scalar1=ad_own[:, HEADS * b + h:HEADS * b + h + 1],
                    scalar2=None, op0=mybir.AluOpType.add)
                l1 = wk.tile([P, kmax], F32, name="l1", tag="l1")
                nc.vector.tensor_scalar(out=l1[:, :kg], in0=eh[:, :kg],
                                        scalar1=0.0, scalar2=NEG,
                                        op0=mybir.AluOpType.min,
                                        op1=mybir.AluOpType.mult)
                nc.vector.tensor_scalar(out=eh[:, :kg], in0=eh[:, :kg],
                                        scalar1=0.0, scalar2=None,
                                        op0=mybir.AluOpType.max)
                nc.vector.tensor_tensor(out=eh[:, :kg], in0=eh[:, :kg],
                                        in1=l1[:, :kg],
                                        op=mybir.AluOpType.add)
                ph = wk.tile([P, kmax], F32, name="ph", tag="ph")
                nc.scalar.activation(out=ph[:, :kg], in_=eh[:, :kg],
                                     func=mybir.ActivationFunctionType.Exp)
                nc.vector.tensor_tensor(out=ph[:, :kg], in0=ph[:, :kg],
                                        in1=svalid[:, c0:c0 + kg],
                                        op=mybir.AluOpType.mult)
                ssh = wk.tile([P, 1], F32, name="ssh", tag="ssh")
                nc.vector.reduce_sum(out=ssh[:], in_=ph[:, :kg],
                                     axis=mybir.AxisListType.X)
                # weighted feature sum over slots: broadcast ph along feature axis
                ph_ap = ph[:, :kg]
                ph_b = bass.AP(tensor=ph_ap.tensor, offset=ph_ap.offset,
                               ap=[list(a) for a in ph_ap.ap] + [[0, H]])
                wf = wk.tile([P, kmax, H], F32, name="wf", tag="wf")
                nc.vector.tensor_tensor(out=wf[:, :kg, :],
                                        in0=g[:, :kg, h * H:(h + 1) * H],
                                        in1=ph_b, op=mybir.AluOpType.mult)
                x3h = wk.tile([P, H], F32, name="x3h", tag=f"x3h{h}")
                nc.vector.reduce_sum(
                    out=x3h[:], in_=wf[:, :kg, :].rearrange("p k f -> p f k"),
                    axis=mybir.AxisListType.X)
                # scale by 1/(2*ssum)
                s2 = wk.tile([P, 1], F32, name="s2", tag="s2")
                nc.vector.tensor_scalar(out=s2[:], in0=ssh[:], scalar1=2.0,
                                        scalar2=1e-20,
                                        op0=mybir.AluOpType.mult,
                                        op1=mybir.AluOpType.max)
                ri = wk.tile([P, 1], F32, name="ri", tag="ri")
                nc.vector.reciprocal(out=ri[:], in_=s2[:])
                nc.vector.tensor_scalar(out=x3h[:], in0=x3h[:], scalar1=ri[:],
                                        scalar2=None,
                                        op0=mybir.AluOpType.mult)
                hx.append(x3h)
            x3c = x3_own[:, b * H:(b + 1) * H]
            nc.vector.tensor_tensor(out=x3c, in0=hx[0][:], in1=hx[1][:],
                                    op=mybir.AluOpType.add)
            nc.vector.tensor_tensor(out=x3c, in0=x3c, in1=bgat_t[:],
                                    op=mybir.AluOpType.add)
            block_stats(x3c, x3c, st3_acc, b)

        if dbg:
            nc.sync.dma_start(out=dbg_d["dbg_xg"][:], in_=xg_c[:])
            nc.sync.dma_start(out=dbg_d["dbg_ad"][:], in_=ad_own[:])
            nc.sync.dma_start(out=dbg_d["dbg_x3"][:], in_=x3_own[:])
        st3 = stats_allreduce(st3_acc, "3")
        a3_t, c3_t = bn_coeffs(st3, w_sb["g3"][:], w_sb["be3"][:], ntot, "3")

        # ================= phase N: BN3 + pooling =================
        pool_acc = cst.tile([B, H + 1], F32, name="pool_acc")
        nc.vector.memset(pool_acc[:], 0.0)
        for b in range(nblk):
            x3c = x3_own[:, b * H:(b + 1) * H]
            nc.vector.tensor_tensor(out=x3c, in0=x3c, in1=a3_t[:],
                                    op=mybir.AluOpType.mult)
            nc.vector.tensor_tensor(out=x3c, in0=x3c, in1=c3_t[:],
                                    op=mybir.AluOpType.add)
            nc.vector.tensor_scalar(out=x3c, in0=x3c, scalar1=0.0,
                                    scalar2=None, op0=mybir.AluOpType.max)
            mg = wk.tile([P, B], F32, name="mg")
            nc.vector.tensor_scalar(out=mg[:], in0=iota_f[:],
                                    scalar1=bcols_sb[:, b:b + 1],
                                    scalar2=None, op0=mybir.AluOpType.is_equal)
            pool_ps = ps_st.tile([B, H + 1], F32, tag="st", name="pool_ps")
            nc.tensor.matmul(out=pool_ps[:, 0:H], lhsT=mg[:], rhs=x3c,
                             start=True, stop=True)
            nc.tensor.matmul(out=pool_ps[:, H:H + 1], lhsT=mg[:],
                             rhs=ones_col[:], start=True, stop=True)
            nc.vector.tensor_tensor(out=pool_acc[:], in0=pool_acc[:],
                                    in1=pool_ps[:], op=mybir.AluOpType.add)

        if dbg:
            nc.sync.dma_start(out=dbg_d["dbg_pool"][:], in_=pool_acc[:])
        pool_c = dr.tile([B, H + 1], F32, name="pool_c")
        pool_f = dr.tile([B, H + 1], F32, name="pool_f", addr_space="Shared")
        nc.sync.dma_start(out=pool_c[:], in_=pool_acc[:])
        nc.gpsimd.collective_compute(
            "AllReduce", mybir.AluOpType.add, replica_groups=grp,
            ins=[pool_c[:]], outs=[pool_f[:]])
        p_sb = cst.tile([B, H + 1], F32, name="p_sb")
        nc.sync.dma_start(out=p_sb[:], in_=pool_f[:])

        # ================= phase P: head (replicated) =================
        rn = cst.tile([B, 1], F32, name="rn")
        nc.vector.reciprocal(out=rn[:], in_=p_sb[:, H:H + 1])
        xc = cst.tile([B, 2 * H], F32, name="xc")
        nc.vector.tensor_scalar(out=xc[:, 0:H], in0=p_sb[:, 0:H],
                                scalar1=rn[:], scalar2=None,
                                op0=mybir.AluOpType.mult)
        nc.vector.tensor_copy(out=xc[:, H:2 * H], in_=p_sb[:, 0:H])
        xcT_ps = ps_tp.tile([2 * H, B], F32, tag="tp", name="xcT_ps")
        nc.tensor.transpose(out=xcT_ps[:], in_=xc[:], identity=ident[:B, :B])
        xcT = cst.tile([2 * H, B], F32, name="xcT")
        nc.vector.tensor_copy(out=xcT[:], in_=xcT_ps[:])
        z_ps = ps_mm.tile([B, H], F32, tag="mm", name="z_ps")
        nc.tensor.matmul(out=z_ps[:], lhsT=xcT[:], rhs=w_sb["W_m1"][:],
                         start=True, stop=True)
        bm1_t = bcast_row(w_sb["b_m1"][:], H, "bm1")  # [P, H], use first B rows
        z = cst.tile([B, H], F32, name="z")
        nc.vector.tensor_tensor(out=z[:], in0=z_ps[:], in1=bm1_t[:B, :],
                                op=mybir.AluOpType.add)
        # head BN over B graphs (two-pass)
        srow_ps = ps_st.tile([1, H], F32, tag="st", name="srow_ps")
        nc.tensor.matmul(out=srow_ps[:], lhsT=ones_col[:B], rhs=z[:],
                         start=True, stop=True)
        mu_m = cst.tile([1, H], F32, name="mu_m")
        nc.vector.tensor_scalar(out=mu_m[:], in0=srow_ps[:], scalar1=1.0 / B,
                                scalar2=None, op0=mybir.AluOpType.mult)
        mu_t = bcast_row(mu_m[:], H, "mu_m")
        zc = cst.tile([B, H], F32, name="zc")
        nc.vector.tensor_tensor(out=zc[:], in0=z[:], in1=mu_t[:B, :],
                                op=mybir.AluOpType.subtract)
        zsq = cst.tile([B, H], F32, name="zsq")
        nc.scalar.activation(out=zsq[:], in_=zc[:],
                             func=mybir.ActivationFunctionType.Square)
        s2_ps = ps_st.tile([1, H], F32, tag="st", name="s2_ps")
        nc.tensor.matmul(out=s2_ps[:], lhsT=ones_col[:B], rhs=zsq[:],
                         start=True, stop=True)
        var_m = cst.tile([1, H], F32, name="var_m")
        nc.vector.tensor_scalar(out=var_m[:], in0=s2_ps[:], scalar1=1.0 / B,
                                scalar2=None, op0=mybir.AluOpType.mult)
        std_m = cst.tile([1, H], F32, name="std_m")
        nc.scalar.activation(out=std_m[:], in_=var_m[:],
                             func=mybir.ActivationFunctionType.Sqrt,
                             bias=eps_sb[0:1, 0:1], scale=1.0)
        rstd_m = cst.tile([1, H], F32, name="rstd_m")
        nc.vector.reciprocal(out=rstd_m[:], in_=std_m[:])
        am_row = cst.tile([1, H], F32, name="am_row")
        nc.vector.tensor_tensor(out=am_row[:], in0=w_sb["gm"][:],
                                in1=rstd_m[:], op=mybir.AluOpType.mult)
        am_t = bcast_row(am_row[:], H, "am")
        bem_t = bcast_row(w_sb["bem"][:], H, "bem")
        hh = cst.tile([B, H], F32, name="hh")
        nc.vector.tensor_tensor(out=hh[:], in0=zc[:], in1=am_t[:B, :],
                                op=mybir.AluOpType.mult)
        nc.vector.tensor_tensor(out=hh[:], in0=hh[:], in1=bem_t[:B, :],
                                op=mybir.AluOpType.add)
        nc.vector.tensor_scalar(out=hh[:], in0=hh[:], scalar1=0.0,
                                scalar2=None, op0=mybir.AluOpType.max)
        hT_ps = ps_tp.tile([H, B], F32, tag="tp", name="hT_ps")
        nc.tensor.transpose(out=hT_ps[:], in_=hh[:], identity=ident[:B, :B])
        hT = cst.tile([H, B], F32, name="hT")
        nc.vector.tensor_copy(out=hT[:], in_=hT_ps[:])
        o_ps = ps_mm.tile([B, OUT], F32, tag="mm", name="o_ps")
        nc.tensor.matmul(out=o_ps[:], lhsT=hT[:], rhs=w_sb["W_m2"][:],
                         start=True, stop=True)
        bm2_t = bcast_row(w_sb["b_m2"][:], OUT, "bm2")
        o_sb = cst.tile([B, OUT], F32, name="o_sb")
        nc.vector.tensor_tensor(out=o_sb[:], in0=o_ps[:], in1=bm2_t[:B, :],
                                op=mybir.AluOpType.add)
        nc.sync.dma_start(out=out_d[:], in_=o_sb[:])

    nc.compile()
    return nc


# --------------------------------------------------------------------------
# entry point
# --------------------------------------------------------------------------

def make_in_maps(inputs, cfg, xt_all, idx_all, bcols_all):
    w = {k: np.ascontiguousarray(np.asarray(v, np.float32))
         for k, v in inputs.items()
         if k not in ("x", "edge_index", "batch")}
    hh = w["att_src"].shape[1]
    shared = {
        "W_gcn": w["W_gcn"], "W_sl": w["W_sl"], "W_sr": w["W_sr"],
        "W_gat": w["W_gat"], "W_m1": w["W_m1"], "W_m2": w["W_m2"],
        "b_gcn": w["b_gcn"].reshape(1, -1), "b_sage": w["b_sage"].reshape(1, -1),
        "b_gat": w["b_gat"].reshape(1, -1),
        "att_s": w["att_src"].reshape(1, -1), "att_d": w["att_dst"].reshape(1, -1),
        "g1": w["g1"].reshape(1, -1), "be1": w["be1"].reshape(1, -1),
        "g2": w["g2"].reshape(1, -1), "be2": w["be2"].reshape(1, -1),
        "g3": w["g3"].reshape(1, -1), "be3": w["be3"].reshape(1, -1),
        "gm": w["gm"].reshape(1, -1), "bem": w["bem"].reshape(1, -1),
        "b_m1": w["b_m1"].reshape(1, -1), "b_m2": w["b_m2"].reshape(1, -1),
    }
    in_maps = []
    for c in range(cfg["ncores"]):
        m = dict(shared)
        m["xT"] = np.ascontiguousarray(xt_all[c])
        m["idx"] = np.ascontiguousarray(idx_all[c])
        m["bcols"] = np.ascontiguousarray(bcols_all[c])
        in_maps.append(m)
    return in_maps


def kernel(**inputs):
    x = np.asarray(inputs["x"], np.float32)
    cfg, xt_all, idx_all, bcols_all = host_prep(
        x, np.asarray(inputs["edge_index"]), np.asarray(inputs["batch"]),
        NCORES)
    nc = build_program(cfg)
    in_maps = make_in_maps(inputs, cfg, xt_all, idx_all, bcols_all)
    res = run_bass_kernel_spmd(nc, in_maps, core_ids=list(range(NCORES)))
    return np.asarray(res.results[0]["out"], np.float32)


# revision 12
# speedup vs baseline: 1.0053x; 1.0053x over previous
"""Trainium2 Bass kernel for NeuronGCN (GCN -> SAGE -> GAT -> pool -> MLP head).

Sharding: nodes partitioned across 8 cores (12500 each), edges partitioned by
destination node. Within each core, owned nodes are sorted by in-degree
(descending) and packed into 128-row blocks; each node's in-edges occupy "slot"
columns of its block (slot k of block b lives at a shared column index), padded
with a pointer to a known all-zero table row. Per-edge work becomes:
one multi-row indirect DMA gather per block + free-axis reductions. Weights are
replicated; BN statistics and pooled per-graph sums are AllReduced; per-node
feature tables (gather sources) are AllGathered between layers.

All floating-point math runs on device. The host only does integer graph
layout (sharding, degree-sort permutation, slot/column assignment, dtype
casts) and data movement.
"""

import sys

sys.path.insert(0, "/opt/trn_rl_repo")

import numpy as np

import concourse.bass as bass
import concourse.bacc as bacc
import concourse.tile as tile
from concourse import mybir
from concourse.masks import make_identity
from concourse.bass_utils import run_bass_kernel_spmd

F32 = mybir.dt.float32
I32 = mybir.dt.int32

# problem constants
N = 100000
E = 800000
IN = 128
H = 64
HEADS = 2
B = 64
OUT = 10
EPS = 1e-5
NEG = 0.2
NCORES = 8
P = 128


# --------------------------------------------------------------------------
# host-side graph layout (integer/index work only)
# --------------------------------------------------------------------------

def host_prep(x, edge_index, batch, ncores):
    n = x.shape[0]
    npc = n // ncores
    assert npc * ncores == n and npc % P != 0
    npad = ((npc + P - 1) // P) * P
    nblk = npad // P
    vlast = npc - P * (nblk - 1)

    src = np.asarray(edge_index[0], dtype=np.int64)
    dst = np.asarray(edge_index[1], dtype=np.int64)
    batch = np.asarray(batch, dtype=np.int64)
    indeg = np.bincount(dst, minlength=n)

    # per-core node permutation: owned nodes sorted by in-degree descending
    gid = np.empty(n, dtype=np.int64)          # orig node -> global permuted row
    perms = []
    for c in range(ncores):
        vs = np.arange(c * npc, (c + 1) * npc)
        order = np.argsort(-indeg[vs], kind="stable")
        pc = vs[order]
        perms.append(pc)
        gid[pc] = c * npad + np.arange(npc)

    # shared slot structure: Kb[b] = max in-degree within block b across cores
    kb = np.zeros(nblk, dtype=np.int64)
    for c in range(ncores):
        d = indeg[perms[c]]
        d = np.concatenate([d, np.zeros(npad - npc, np.int64)])
        kb = np.maximum(kb, d.reshape(nblk, P).max(axis=1))
    colstart = np.zeros(nblk + 1, dtype=np.int64)
    colstart[1:] = np.cumsum(kb + 1)           # +1 self slot per block
    s_total = int(colstart[-1])
    zrow = npad - 1                            # core-0 pad row: all-zero in every table

    owner = dst // npc
    idx_all = np.full((ncores, P, s_total), zrow, dtype=np.int32)
    for c in range(ncores):
        m = owner == c
        es, ed = src[m], dst[m]
        lpos = gid[ed] - c * npad              # local slot position [0, npc)
        o = np.argsort(lpos, kind="stable")
        lpos, es = lpos[o], es[o]
        counts = np.bincount(lpos, minlength=npc)
        starts = np.concatenate([[0], np.cumsum(counts)[:-1]])
        k = np.arange(len(lpos)) - starts[lpos]
        bb, pp = lpos // P, lpos % P
        idx_all[c, pp, colstart[bb] + k] = gid[es].astype(np.int32)
        # self slot (last column of each block's range) for real nodes
        lp = np.arange(npc)
        idx_all[c, lp % P, colstart[lp // P] + kb[lp // P]] = (
            c * npad + lp
        ).astype(np.int32)

    xt_all = np.zeros((ncores, x.shape[1], npad), dtype=np.float32)
    bcols_all = np.full((ncores, P, nblk), float(B), dtype=np.float32)
    for c in range(ncores):
        xt_all[c, :, :npc] = x[perms[c]].T
        bc = np.full(npad, B, np.int64)
        bc[:npc] = batch[perms[c]]
        bcols_all[c] = bc.reshape(nblk, P).T.astype(np.float32)

    cfg = dict(
        ncores=ncores, npc=npc, npad=npad, nblk=nblk, vlast=vlast,
        kb=[int(v) for v in kb], colstart=[int(v) for v in colstart],
        s_total=s_total, zrow=zrow, ntot=n, in_dim=x.shape[1],
    )
    return cfg, xt_all, idx_all, bcols_all


# --------------------------------------------------------------------------
# device program
# --------------------------------------------------------------------------

def build_program(cfg, dbg=False, single=False):
    ncores = cfg["ncores"]
    npad, nblk, vlast = cfg["npad"], cfg["nblk"], cfg["vlast"]
    kb, colstart, s_total = cfg["kb"], cfg["colstart"], cfg["s_total"]
    zrow, ntot, in_dim = cfg["zrow"], cfg["ntot"], cfg["in_dim"]
    kmax = max(kb) + 1
    grp = [list(range(ncores))]
    SHARED = "Local" if single else "Shared"

    nc = bacc.Bacc("TRN2", target_bir_lowering=False, debug=False,
                   num_devices=1 if single else ncores)

    # I/O
    xT_d = nc.dram_tensor("xT", [in_dim, npad], F32, kind="ExternalInput")
    idx_d = nc.dram_tensor("idx", [P, s_total], I32, kind="ExternalInput")
    bcols_d = nc.dram_tensor("bcols", [P, nblk], F32, kind="ExternalInput")
    wnames = {
        "W_gcn": [in_dim, H], "W_sl": [H, H], "W_sr": [H, H],
        "W_gat": [H, HEADS * H], "W_m1": [2 * H, H], "W_m2": [H, OUT],
        "b_gcn": [1, H], "b_sage": [1, H], "b_gat": [1, H],
        "att_s": [1, HEADS * H], "att_d": [1, HEADS * H],
        "g1": [1, H], "be1": [1, H], "g2": [1, H], "be2": [1, H],
        "g3": [1, H], "be3": [1, H], "gm": [1, H], "bem": [1, H],
        "b_m1": [1, H], "b_m2": [1, OUT],
    }
    w_d = {k: nc.dram_tensor(k, shp, F32, kind="ExternalInput")
           for k, shp in wnames.items()}
    out_d = nc.dram_tensor("out", [B, OUT], F32, kind="ExternalOutput")
    GWc = HEADS * H + HEADS
    if dbg:
        dbg_d = {
            "dbg_deg": nc.dram_tensor("dbg_deg", [P, nblk], F32, kind="ExternalOutput"),
            "dbg_dinv": nc.dram_tensor("dbg_dinv", [P, nblk], F32, kind="ExternalOutput"),
            "dbg_x1": nc.dram_tensor("dbg_x1", [P, nblk * H], F32, kind="ExternalOutput"),
            "dbg_st1": nc.dram_tensor("dbg_st1", [1, 2 * H], F32, kind="ExternalOutput"),
            "dbg_x1n": nc.dram_tensor("dbg_x1n", [npad, H], F32, kind="ExternalOutput"),
            "dbg_x2": nc.dram_tensor("dbg_x2", [P, nblk * H], F32, kind="ExternalOutput"),
            "dbg_xg": nc.dram_tensor("dbg_xg", [npad, GWc], F32, kind="ExternalOutput"),
            "dbg_ad": nc.dram_tensor("dbg_ad", [P, nblk * HEADS], F32, kind="ExternalOutput"),
            "dbg_x3": nc.dram_tensor("dbg_x3", [P, nblk * H], F32, kind="ExternalOutput"),
            "dbg_pool": nc.dram_tensor("dbg_pool", [B, H + 1], F32, kind="ExternalOutput"),
        }

    from contextlib import ExitStack
    with tile.TileContext(nc) as tc, ExitStack() as stk:
        cst = stk.enter_context(tc.tile_pool(name="cst", bufs=1))
        wk = stk.enter_context(tc.tile_pool(name="wk", bufs=2))
        gp = stk.enter_context(tc.tile_pool(name="gp", bufs=2))
        ps_mm = stk.enter_context(tc.tile_pool(name="ps_mm", bufs=2, space="PSUM"))
        ps_tp = stk.enter_context(tc.tile_pool(name="ps_tp", bufs=2, space="PSUM"))
        ps_st = stk.enter_context(tc.tile_pool(name="ps_st", bufs=2, space="PSUM"))
        dr = stk.enter_context(tc.tile_pool(name="dr", bufs=1, space="DRAM"))

        nc._phase_marks = []

        def mark(name):
            nc._phase_marks.append((name, len(nc.instructions)
                                    if hasattr(nc, "instructions")
                                    else nc.next_id()))

        mark("consts")
        # ---- constants ----
        ident = cst.tile([P, P], F32, name="ident")
        make_identity(nc, ident[:])
        ones_row = cst.tile([1, P], F32, name="ones_row")
        nc.vector.memset(ones_row[:], 1.0)
        ones_col = cst.tile([P, 1], F32, name="ones_col")
        nc.vector.memset(ones_col[:], 1.0)
        eps_sb = cst.tile([1, 1], F32, name="eps_sb")
        nc.vector.memset(eps_sb[:], EPS)
        iota_i = cst.tile([P, B], I32, name="iota_i")
        nc.gpsimd.iota(iota_i[:], pattern=[[1, B]], base=0, channel_multiplier=0)
        iota_f = cst.tile([P, B], F32, name="iota_f")
        nc.vector.tensor_copy(out=iota_f[:], in_=iota_i[:])
        pidx_i = cst.tile([P, 1], I32, name="pidx_i")
        nc.gpsimd.iota(pidx_i[:], pattern=[[1, 1]], base=0, channel_multiplier=1)
        pidx_f = cst.tile([P, 1], F32, name="pidx_f")
        nc.vector.tensor_copy(out=pidx_f[:], in_=pidx_i[:])
        vmask = cst.tile([P, 1], F32, name="vmask")
        nc.vector.tensor_scalar(out=vmask[:], in0=pidx_f[:],
                                scalar1=float(vlast), scalar2=None,
                                op0=mybir.AluOpType.is_lt)

        w_sb = {}
        for k, shp in wnames.items():
            w_sb[k] = cst.tile(shp, F32, name=f"w_{k}")
            nc.sync.dma_start(out=w_sb[k][:], in_=w_d[k][:])

        idx_sb = cst.tile([P, s_total], I32, name="idx_sb")
        nc.sync.dma_start(out=idx_sb[:], in_=idx_d[:])
        bcols_sb = cst.tile([P, nblk], F32, name="bcols_sb")
        nc.sync.dma_start(out=bcols_sb[:], in_=bcols_d[:])

        def bcast_row(row_ap, width, name):
            """materialize [P, width] tile with every row = row_ap ([1, width])"""
            t_ps = ps_mm.tile([P, width], F32, tag="mm", name=f"bc_{name}_ps")
            nc.tensor.matmul(out=t_ps[:], lhsT=ones_row[:], rhs=row_ap,
                             start=True, stop=True)
            t = cst.tile([P, width], F32, name=f"bc_{name}")
            nc.vector.tensor_copy(out=t[:], in_=t_ps[:])
            return t

        bgcn_t = bcast_row(w_sb["b_gcn"][:], H, "bgcn")
        bsage_t = bcast_row(w_sb["b_sage"][:], H, "bsage")
        bgat_t = bcast_row(w_sb["b_gat"][:], H, "bgat")
        atts_t = bcast_row(w_sb["att_s"][:], HEADS * H, "atts")
        attd_t = bcast_row(w_sb["att_d"][:], HEADS * H, "attd")

        # ---- residents ----
        x1_own = cst.tile([P, nblk * H], F32, name="x1_own")
        x2_own = cst.tile([P, nblk * H], F32, name="x2_own")
        x3_own = cst.tile([P, nblk * H], F32, name="x3_own")
        ad_own = cst.tile([P, nblk * HEADS], F32, name="ad_own")
        dinv_sb = cst.tile([P, nblk], F32, name="dinv_sb")
        cntinv_sb = cst.tile([P, nblk], F32, name="cntinv_sb")

        # DRAM tables + bounce buffers
        xws_c = dr.tile([npad, H], F32, name="xws_c")
        x1n_c = dr.tile([npad, H], F32, name="x1n_c")
        xg_c = dr.tile([npad, HEADS * H + HEADS], F32, name="xg_c")
        xws_full = dr.tile([ncores * npad, H], F32, name="xws_full",
                           addr_space=SHARED)
        x1n_full = dr.tile([ncores * npad, H], F32, name="x1n_full",
                           addr_space=SHARED)
        xg_full = dr.tile([ncores * npad, HEADS * H + HEADS], F32,
                          name="xg_full", addr_space=SHARED)
        GW = HEADS * H + HEADS  # packed gat row width: xg(128) | a_s(2)

        def stats_allreduce(acc_sb, name):
            """AllReduce a [1, 2H] stats row; returns sbuf [1, 2H]."""
            st_c = dr.tile([1, 2 * H], F32, name=f"st_{name}_c")
            st_f = dr.tile([1, 2 * H], F32, name=f"st_{name}_f",
                           addr_space=SHARED)
            nc.sync.dma_start(out=st_c[:], in_=acc_sb[:])
            if single:
                nc.sync.dma_start(out=st_f[:], in_=st_c[:])
            else:
                nc.gpsimd.collective_compute(
                    "AllReduce", mybir.AluOpType.add, replica_groups=grp,
                    ins=[st_c[:]], outs=[st_f[:]])
            st_sb = cst.tile([1, 2 * H], F32, name=f"st_{name}_sb")
            nc.sync.dma_start(out=st_sb[:], in_=st_f[:])
            return st_sb

        def bn_coeffs(st_sb, g_row, be_row, count, name):
            """From [1,2H] (sum|sumsq) rows: A = g*rstd, C = be - mu*A,
            materialized as [P, H] tiles."""
            mu = cst.tile([1, H], F32, name=f"mu_{name}")
            nc.vector.tensor_scalar(out=mu[:], in0=st_sb[:, 0:H],
                                    scalar1=1.0 / count, scalar2=None,
                                    op0=mybir.AluOpType.mult)
            ex2 = cst.tile([1, H], F32, name=f"ex2_{name}")
            nc.vector.tensor_scalar(out=ex2[:], in0=st_sb[:, H:2 * H],
                                    scalar1=1.0 / count, scalar2=None,
                                    op0=mybir.AluOpType.mult)
            musq = cst.tile([1, H], F32, name=f"musq_{name}")
            nc.vector.tensor_tensor(out=musq[:], in0=mu[:], in1=mu[:],
                                    op=mybir.AluOpType.mult)
            var = cst.tile([1, H], F32, name=f"var_{name}")
            nc.vector.tensor_tensor(out=var[:], in0=ex2[:], in1=musq[:],
                                    op=mybir.AluOpType.subtract)
            std = cst.tile([1, H], F32, name=f"std_{name}")
            nc.scalar.activation(out=std[:], in_=var[:],
                                 func=mybir.ActivationFunctionType.Sqrt,
                                 bias=eps_sb[0:1, 0:1], scale=1.0)
            rstd = cst.tile([1, H], F32, name=f"rstd_{name}")
            nc.vector.reciprocal(out=rstd[:], in_=std[:])
            a_row = cst.tile([1, H], F32, name=f"arow_{name}")
            nc.vector.tensor_tensor(out=a_row[:], in0=g_row, in1=rstd[:],
                                    op=mybir.AluOpType.mult)
            mua = cst.tile([1, H], F32, name=f"mua_{name}")
            nc.vector.tensor_tensor(out=mua[:], in0=mu[:], in1=a_row[:],
                                    op=mybir.AluOpType.mult)
            c_row = cst.tile([1, H], F32, name=f"crow_{name}")
            nc.vector.tensor_tensor(out=c_row[:], in0=be_row, in1=mua[:],
                                    op=mybir.AluOpType.subtract)
            return bcast_row(a_row[:], H, f"A_{name}"), \
                bcast_row(c_row[:], H, f"C_{name}")

        def block_stats(x_ap, sq_src, acc_sb, b):
            """acc_sb[0,0:H] += col-sums of x, acc_sb[0,H:2H] += col-sums of x^2
            over valid rows of block b."""
            v = vlast if b == nblk - 1 else P
            sq = wk.tile([P, H], F32, name="sq_st")
            nc.scalar.activation(out=sq[:v], in_=sq_src[:v],
                                 func=mybir.ActivationFunctionType.Square)
            st_ps = ps_st.tile([1, 2 * H], F32, tag="st", name="st_ps")
            nc.tensor.matmul(out=st_ps[:, 0:H], lhsT=ones_col[:v],
                             rhs=x_ap[:v], start=True, stop=True)
            nc.tensor.matmul(out=st_ps[:, H:2 * H], lhsT=ones_col[:v],
                             rhs=sq[:v], start=True, stop=True)
            nc.vector.tensor_tensor(out=acc_sb[:], in0=acc_sb[:],
                                    in1=st_ps[:], op=mybir.AluOpType.add)

        def transpose_sb(src_ap, pdim, fdim, name):
            """PE transpose [pdim, fdim] -> sbuf [fdim, pdim]."""
            t_ps = ps_tp.tile([fdim, pdim], F32, tag="tp", name=f"{name}_ps")
            nc.tensor.transpose(out=t_ps[:], in_=src_ap,
                                identity=ident[:pdim, :pdim])
            t_sb = wk.tile([fdim, pdim], F32, name=f"{name}_sb", tag=name)
            nc.vector.tensor_copy(out=t_sb[:], in_=t_ps[:])
            return t_sb

        mark("A_deg")
        # ================= phase A: slot validity + degrees =================
        svalid = cst.tile([P, s_total], F32, name="svalid")
        nc.vector.tensor_copy(out=svalid[:], in_=idx_sb[:])
        nc.vector.tensor_scalar(out=svalid[:], in0=svalid[:],
                                scalar1=float(zrow), scalar2=None,
                                op0=mybir.AluOpType.not_equal)
        for b in range(nblk):
            k = kb[b]
            if k == 0:
                nc.vector.memset(dinv_sb[:, b:b + 1], 0.0)  # deg=0 marker
                continue
            c0 = colstart[b]
            nc.vector.reduce_sum(out=dinv_sb[:, b:b + 1],
                                 in_=svalid[:, c0:c0 + k],
                                 axis=mybir.AxisListType.X)
        # dinv_sb currently holds deg; derive cntinv = 1/max(deg,1), dinv = 1/sqrt(deg+1)
        if dbg:
            nc.sync.dma_start(out=dbg_d["dbg_deg"][:], in_=dinv_sb[:])
        tmp_deg = cst.tile([P, nblk], F32, name="tmp_deg")
        nc.vector.tensor_scalar(out=tmp_deg[:], in0=dinv_sb[:], scalar1=1.0,
                                scalar2=None, op0=mybir.AluOpType.max)
        nc.vector.reciprocal(out=cntinv_sb[:], in_=tmp_deg[:])
        sq_deg = cst.tile([P, nblk], F32, name="sq_deg")
        nc.scalar.activation(out=sq_deg[:], in_=dinv_sb[:],
                             func=mybir.ActivationFunctionType.Sqrt,
                             bias=1.0, scale=1.0)
        nc.vector.reciprocal(out=dinv_sb[:], in_=sq_deg[:])
        if dbg:
            nc.sync.dma_start(out=dbg_d["dbg_dinv"][:], in_=dinv_sb[:])

        mark("B_xws")
        # ================= phase B: xws table =================
        for b in range(nblk):
            xtb = wk.tile([in_dim, P], F32, name="xtb")
            nc.sync.dma_start(out=xtb[:], in_=xT_d[:, b * P:(b + 1) * P])
            xw_ps = ps_mm.tile([P, H], F32, tag="mm", name="xw_ps")
            nc.tensor.matmul(out=xw_ps[:], lhsT=xtb[:], rhs=w_sb["W_gcn"][:],
                             start=True, stop=True)
            xws = wk.tile([P, H], F32, name="xws")
            nc.vector.tensor_scalar(out=xws[:], in0=xw_ps[:],
                                    scalar1=dinv_sb[:, b:b + 1], scalar2=None,
                                    op0=mybir.AluOpType.mult)
            nc.sync.dma_start(out=xws_c[b * P:(b + 1) * P, :], in_=xws[:])

        if single:
            for _c in range(ncores):
                nc.sync.dma_start(out=xws_full[_c * npad:(_c + 1) * npad, :],
                                  in_=xws_c[:])
        else:
            nc.gpsimd.collective_compute(
                "AllGather", mybir.AluOpType.bypass, replica_groups=grp,
                ins=[xws_c[:]], outs=[xws_full[:]])

        mark("D_gcn_agg")
        # ================= phase D: GCN aggregate =================
        st1_acc = cst.tile([1, 2 * H], F32, name="st1_acc")
        nc.vector.memset(st1_acc[:], 0.0)
        for b in range(nblk):
            kg = kb[b] + 1
            c0 = colstart[b]
            g = gp.tile([P, kmax, H], F32, name="g_gcn", tag="g_gcn")
            for k in range(kg):
                nc.gpsimd.indirect_dma_start(
                    out=g[:, k, :], out_offset=None, in_=xws_full[:],
                    in_offset=bass.IndirectOffsetOnAxis(
                        ap=idx_sb[:, c0 + k:c0 + k + 1], axis=0))
            agg = wk.tile([P, H], F32, name="agg_gcn")
            nc.vector.reduce_sum(
                out=agg[:], in_=g[:, :kg, :].rearrange("p k f -> p f k"),
                axis=mybir.AxisListType.X)
            x1c = x1_own[:, b * H:(b + 1) * H]
            nc.vector.tensor_scalar(out=agg[:], in0=agg[:],
                                    scalar1=dinv_sb[:, b:b + 1], scalar2=None,
                                    op0=mybir.AluOpType.mult)
            nc.vector.tensor_tensor(out=x1c, in0=agg[:], in1=bgcn_t[:],
                                    op=mybir.AluOpType.add)
            block_stats(x1c, x1c, st1_acc, b)

        mark("F_bn1_write")
        # BN1 -> x1n (in place) -> table
        if dbg:
            nc.sync.dma_start(out=dbg_d["dbg_x1"][:], in_=x1_own[:])
        st1 = stats_allreduce(st1_acc, "1")
        if dbg:
            nc.sync.dma_start(out=dbg_d["dbg_st1"][:], in_=st1[:])
        a1_t, c1_t = bn_coeffs(st1, w_sb["g1"][:], w_sb["be1"][:], ntot, "1")
        for b in range(nblk):
            x1c = x1_own[:, b * H:(b + 1) * H]
            nc.vector.tensor_tensor(out=x1c, in0=x1c, in1=a1_t[:],
                                    op=mybir.AluOpType.mult)
            nc.vector.tensor_tensor(out=x1c, in0=x1c, in1=c1_t[:],
                                    op=mybir.AluOpType.add)
            nc.vector.tensor_scalar(out=x1c, in0=x1c, scalar1=0.0,
                                    scalar2=None, op0=mybir.AluOpType.max)
            if b == nblk - 1:
                nc.vector.tensor_scalar(out=x1c, in0=x1c, scalar1=vmask[:],
                                        scalar2=None, op0=mybir.AluOpType.mult)
            nc.sync.dma_start(out=x1n_c[b * P:(b + 1) * P, :], in_=x1c)

        if single:
            for _c in range(ncores):
                nc.sync.dma_start(out=x1n_full[_c * npad:(_c + 1) * npad, :],
                                  in_=x1n_c[:])
        else:
            nc.gpsimd.collective_compute(
                "AllGather", mybir.AluOpType.bypass, replica_groups=grp,
                ins=[x1n_c[:]], outs=[x1n_full[:]])

        mark("H_sage")
        # ================= phase H: SAGE =================
        st2_acc = cst.tile([1, 2 * H], F32, name="st2_acc")
        nc.vector.memset(st2_acc[:], 0.0)
        for b in range(nblk):
            k = kb[b]
            c0 = colstart[b]
            agg = wk.tile([P, H], F32, name="agg_sage")
            if k == 0:
                nc.vector.memset(agg[:], 0.0)
            else:
                g = gp.tile([P, kmax, H], F32, name="g_sage", tag="g_gcn")
                for kk in range(k):
                    nc.gpsimd.indirect_dma_start(
                        out=g[:, kk, :], out_offset=None, in_=x1n_full[:],
                        in_offset=bass.IndirectOffsetOnAxis(
                            ap=idx_sb[:, c0 + kk:c0 + kk + 1], axis=0))
                nc.vector.reduce_sum(
                    out=agg[:], in_=g[:, :k, :].rearrange("p k f -> p f k"),
                    axis=mybir.AxisListType.X)
            nc.vector.tensor_scalar(out=agg[:], in0=agg[:],
                                    scalar1=cntinv_sb[:, b:b + 1],
                                    scalar2=None, op0=mybir.AluOpType.mult)
            aggT = transpose_sb(agg[:], P, H, "aggT")
            x1T = transpose_sb(x1_own[:, b * H:(b + 1) * H], P, H, "x1T")
            x2_ps = ps_mm.tile([P, H], F32, tag="mm", name="x2_ps")
            nc.tensor.matmul(out=x2_ps[:], lhsT=aggT[:], rhs=w_sb["W_sl"][:],
                             start=True, stop=False)
            nc.tensor.matmul(out=x2_ps[:], lhsT=x1T[:], rhs=w_sb["W_sr"][:],
                             start=False, stop=True)
            x2c = x2_own[:, b * H:(b + 1) * H]
            nc.vector.tensor_tensor(out=x2c, in0=x2_ps[:], in1=bsage_t[:],
                                    op=mybir.AluOpType.add)
            block_stats(x2c, x2c, st2_acc, b)

        if dbg:
            nc.sync.dma_start(out=dbg_d["dbg_x1n"][:], in_=x1n_c[:])
            nc.sync.dma_start(out=dbg_d["dbg_x2"][:], in_=x2_own[:])
        st2 = stats_allreduce(st2_acc, "2")
        a2_t, c2_t = bn_coeffs(st2, w_sb["g2"][:], w_sb["be2"][:], ntot, "2")

        mark("J_gat_table")
        # ================= phase J: BN2 + GAT table =================
        for b in range(nblk):
            x2c = x2_own[:, b * H:(b + 1) * H]
            nc.vector.tensor_tensor(out=x2c, in0=x2c, in1=a2_t[:],
                                    op=mybir.AluOpType.mult)
            nc.vector.tensor_tensor(out=x2c, in0=x2c, in1=c2_t[:],
                                    op=mybir.AluOpType.add)
            nc.vector.tensor_scalar(out=x2c, in0=x2c, scalar1=0.0,
                                    scalar2=None, op0=mybir.AluOpType.max)
            if b == nblk - 1:
                nc.vector.tensor_scalar(out=x2c, in0=x2c, scalar1=vmask[:],
                                        scalar2=None, op0=mybir.AluOpType.mult)
            x2T = transpose_sb(x2c, P, H, "x2T")
            xg_ps = ps_mm.tile([P, HEADS * H], F32, tag="mm", name="xg_ps")
            nc.tensor.matmul(out=xg_ps[:], lhsT=x2T[:], rhs=w_sb["W_gat"][:],
                             start=True, stop=True)
            pk = wk.tile([P, GW], F32, name="pk")
            nc.vector.tensor_copy(out=pk[:, 0:HEADS * H], in_=xg_ps[:])
            ta = wk.tile([P, HEADS * H], F32, name="ta")
            nc.vector.tensor_tensor(out=ta[:], in0=xg_ps[:], in1=atts_t[:],
                                    op=mybir.AluOpType.mult)
            for h in range(HEADS):
                nc.vector.reduce_sum(
                    out=pk[:, HEADS * H + h:HEADS * H + h + 1],
                    in_=ta[:, h * H:(h + 1) * H], axis=mybir.AxisListType.X)
            td = wk.tile([P, HEADS * H], F32, name="td")
            nc.vector.tensor_tensor(out=td[:], in0=xg_ps[:], in1=attd_t[:],
                                    op=mybir.AluOpType.mult)
            for h in range(HEADS):
                nc.vector.reduce_sum(
                    out=ad_own[:, HEADS * b + h:HEADS * b + h + 1],
                    in_=td[:, h * H:(h + 1) * H], axis=mybir.AxisListType.X)
            nc.sync.dma_start(out=xg_c[b * P:(b + 1) * P, :], in_=pk[:])

        if single:
            for _c in range(ncores):
                nc.sync.dma_start(out=xg_full[_c * npad:(_c + 1) * npad, :],
                                  in_=xg_c[:])
        else:
            nc.gpsimd.collective_compute(
                "AllGather", mybir.AluOpType.bypass, replica_groups=grp,
                ins=[xg_c[:]], outs=[xg_full[:]])

        mark("L_gat_agg")
        # ================= phase L: GAT aggregate =================
        st3_acc = cst.tile([1, 2 * H], F32, name="st3_acc")
        nc.vector.memset(st3_acc[:], 0.0)
        for b in range(nblk):
            kg = kb[b] + 1
            c0 = colstart[b]
            g = gp.tile([P, kmax, GW], F32, name="g_gat", tag="g_gat")
            for k in range(kg):
                nc.gpsimd.indirect_dma_start(
                    out=g[:, k, :], out_offset=None, in_=xg_full[:],
                    in_offset=bass.IndirectOffsetOnAxis(
                        ap=idx_sb[:, c0 + k:c0 + k + 1], axis=0))
            hx = []
            for h in range(HEADS):
                eh = wk.tile([P, kmax], F32, name="eh", tag="eh")
                nc.vector.tensor_scalar(
                    out=eh[:, :kg],
                    in0=g[:, :kg, HEADS * H + h:HEADS * H + h + 1],
                    scalar1=ad_own[:, HEADS * b + h:HEADS * b + h + 1],
                    scalar2=None, op0=mybir.AluOpType.add)
                l1 = wk.tile([P, kmax], F32, name="l1", tag="l1")
                nc.vector.tensor_scalar(out=l1[:, :kg], in0=eh[:, :kg],
                                        scalar1=0.0, scalar2=NEG,
                                        op0=mybir.AluOpType.min,
                                        op1=mybir.AluOpType.mult)
                nc.vector.tensor_scalar(out=eh[:, :kg], in0=eh[:, :kg],
                                        scalar1=0.0, scalar2=None,
                                        op0=mybir.AluOpType.max)
                nc.vector.tensor_tensor(out=eh[:, :kg], in0=eh[:, :kg],
                                        in1=l1[:, :kg],
                                        op=mybir.AluOpType.add)
                ph = wk.tile([P, kmax], F32, name="ph", tag="ph")
                nc.scalar.activation(out=ph[:, :kg], in_=eh[:, :kg],
                                     func=mybir.ActivationFunctionType.Exp)
                nc.vector.tensor_tensor(out=ph[:, :kg], in0=ph[:, :kg],
                                        in1=svalid[:, c0:c0 + kg],
                                        op=mybir.AluOpType.mult)
                ssh = wk.tile([P, 1], F32, name="ssh", tag="ssh")
                nc.vector.reduce_sum(out=ssh[:], in_=ph[:, :kg],
                                     axis=mybir.AxisListType.X)
                # weighted feature sum over slots: broadcast ph along feature axis
                ph_ap = ph[:, :kg]
                ph_b = bass.AP(tensor=ph_ap.tensor, offset=ph_ap.offset,
                               ap=[list(a) for a in ph_ap.ap] + [[0, H]])
                wf = wk.tile([P, kmax, H], F32, name="wf", tag="wf")
                nc.vector.tensor_tensor(out=wf[:, :kg, :],
                                        in0=g[:, :kg, h * H:(h + 1) * H],
                                        in1=ph_b, op=mybir.AluOpType.mult)
                x3h = wk.tile([P, H], F32, name="x3h", tag=f"x3h{h}")
                nc.vector.reduce_sum(
                    out=x3h[:], in_=wf[:, :kg, :].rearrange("p k f -> p f k"),
                    axis=mybir.AxisListType.X)
                # scale by 1/(2*ssum)
                s2 = wk.tile([P, 1], F32, name="s2", tag="s2")
                nc.vector.tensor_scalar(out=s2[:], in0=ssh[:], scalar1=2.0,
                                        scalar2=1e-20,
                                        op0=mybir.AluOpType.mult,
                                        op1=mybir.AluOpType.max)
                ri = wk.tile([P, 1], F32, name="ri", tag="ri")
                nc.vector.reciprocal(out=ri[:], in_=s2[:])
                nc.vector.tensor_scalar(out=x3h[:], in0=x3h[:], scalar1=ri[:],
                                        scalar2=None,
                                        op0=mybir.AluOpType.mult)
                hx.append(x3h)
            x3c = x3_own[:, b * H:(b + 1) * H]
            nc.vector.tensor_tensor(out=x3c, in0=hx[0][:], in1=hx[1][:],
                                    op=mybir.AluOpType.add)
            nc.vector.tensor_tensor(out=x3c, in0=x3c, in1=bgat_t[:],
                                    op=mybir.AluOpType.add)
            block_stats(x3c, x3c, st3_acc, b)

        if dbg:
            nc.sync.dma_start(out=dbg_d["dbg_xg"][:], in_=xg_c[:])
            nc.sync.dma_start(out=dbg_d["dbg_ad"][:], in_=ad_own[:])
            nc.sync.dma_start(out=dbg_d["dbg_x3"][:], in_=x3_own[:])
        st3 = stats_allreduce(st3_acc, "3")
        a3_t, c3_t = bn_coeffs(st3, w_sb["g3"][:], w_sb["be3"][:], ntot, "3")

        mark("N_pool")
        # ================= phase N: BN3 + pooling =================
        pool_acc = cst.tile([B, H + 1], F32, name="pool_acc")
        nc.vector.memset(pool_acc[:], 0.0)
        for b in range(nblk):
            x3c = x3_own[:, b * H:(b + 1) * H]
            nc.vector.tensor_tensor(out=x3c, in0=x3c, in1=a3_t[:],
                                    op=mybir.AluOpType.mult)
            nc.vector.tensor_tensor(out=x3c, in0=x3c, in1=c3_t[:],
                                    op=mybir.AluOpType.add)
            nc.vector.tensor_scalar(out=x3c, in0=x3c, scalar1=0.0,
                                    scalar2=None, op0=mybir.AluOpType.max)
            mg = wk.tile([P, B], F32, name="mg")
            nc.vector.tensor_scalar(out=mg[:], in0=iota_f[:],
                                    scalar1=bcols_sb[:, b:b + 1],
                                    scalar2=None, op0=mybir.AluOpType.is_equal)
            pool_ps = ps_st.tile([B, H + 1], F32, tag="st", name="pool_ps")
            nc.tensor.matmul(out=pool_ps[:, 0:H], lhsT=mg[:], rhs=x3c,
                             start=True, stop=True)
            nc.tensor.matmul(out=pool_ps[:, H:H + 1], lhsT=mg[:],
                             rhs=ones_col[:], start=True, stop=True)
            nc.vector.tensor_tensor(out=pool_acc[:], in0=pool_acc[:],
                                    in1=pool_ps[:], op=mybir.AluOpType.add)

        if dbg:
            nc.sync.dma_start(out=dbg_d["dbg_pool"][:], in_=pool_acc[:])
        pool_c = dr.tile([B, H + 1], F32, name="pool_c")
        pool_f = dr.tile([B, H + 1], F32, name="pool_f", addr_space=SHARED)
        nc.sync.dma_start(out=pool_c[:], in_=pool_acc[:])
        if single:
            nc.sync.dma_start(out=pool_f[:], in_=pool_c[:])
        else:
            nc.gpsimd.collective_compute(
                "AllReduce", mybir.AluOpType.add, replica_groups=grp,
                ins=[pool_c[:]], outs=[pool_f[:]])
        p_sb = cst.tile([B, H + 1], F32, name="p_sb")
        nc.sync.dma_start(out=p_sb[:], in_=pool_f[:])

        mark("P_head")
        # ================= phase P: head (replicated) =================
        rn = cst.tile([B, 1], F32, name="rn")
        nc.vector.reciprocal(out=rn[:], in_=p_sb[:, H:H + 1])
        xc = cst.tile([B, 2 * H], F32, name="xc")
        nc.vector.tensor_scalar(out=xc[:, 0:H], in0=p_sb[:, 0:H],
                                scalar1=rn[:], scalar2=None,
                                op0=mybir.AluOpType.mult)
        nc.vector.tensor_copy(out=xc[:, H:2 * H], in_=p_sb[:, 0:H])
        xcT_ps = ps_tp.tile([2 * H, B], F32, tag="tp", name="xcT_ps")
        nc.tensor.transpose(out=xcT_ps[:], in_=xc[:], identity=ident[:B, :B])
        xcT = cst.tile([2 * H, B], F32, name="xcT")
        nc.vector.tensor_copy(out=xcT[:], in_=xcT_ps[:])
        z_ps = ps_mm.tile([B, H], F32, tag="mm", name="z_ps")
        nc.tensor.matmul(out=z_ps[:], lhsT=xcT[:], rhs=w_sb["W_m1"][:],
                         start=True, stop=True)
        bm1_t = bcast_row(w_sb["b_m1"][:], H, "bm1")  # [P, H], use first B rows
        z = cst.tile([B, H], F32, name="z")
        nc.vector.tensor_tensor(out=z[:], in0=z_ps[:], in1=bm1_t[:B, :],
                                op=mybir.AluOpType.add)
        # head BN over B graphs (two-pass)
        srow_ps = ps_st.tile([1, H], F32, tag="st", name="srow_ps")
        nc.tensor.matmul(out=srow_ps[:], lhsT=ones_col[:B], rhs=z[:],
                         start=True, stop=True)
        mu_m = cst.tile([1, H], F32, name="mu_m")
        nc.vector.tensor_scalar(out=mu_m[:], in0=srow_ps[:], scalar1=1.0 / B,
                                scalar2=None, op0=mybir.AluOpType.mult)
        mu_t = bcast_row(mu_m[:], H, "mu_m")
        zc = cst.tile([B, H], F32, name="zc")
        nc.vector.tensor_tensor(out=zc[:], in0=z[:], in1=mu_t[:B, :],
                                op=mybir.AluOpType.subtract)
        zsq = cst.tile([B, H], F32, name="zsq")
        nc.scalar.activation(out=zsq[:], in_=zc[:],
                             func=mybir.ActivationFunctionType.Square)
        s2_ps = ps_st.tile([1, H], F32, tag="st", name="s2_ps")
        nc.tensor.matmul(out=s2_ps[:], lhsT=ones_col[:B], rhs=zsq[:],
                         start=True, stop=True)
        var_m = cst.tile([1, H], F32, name="var_m")
        nc.vector.tensor_scalar(out=var_m[:], in0=s2_ps[:], scalar1=1.0 / B,
                                scalar2=None, op0=mybir.AluOpType.mult)
        std_m = cst.tile([1, H], F32, name="std_m")
        nc.scalar.activation(out=std_m[:], in_=var_m[:],
                             func=mybir.ActivationFunctionType.Sqrt,
                             bias=eps_sb[0:1, 0:1], scale=1.0)
        rstd_m = cst.tile([1, H], F32, name="rstd_m")
        nc.vector.reciprocal(out=rstd_m[:], in_=std_m[:])
        am_row = cst.tile([1, H], F32, name="am_row")
        nc.vector.tensor_tensor(out=am_row[:], in0=w_sb["gm"][:],
                                in1=rstd_m[:], op=mybir.AluOpType.mult)
        am_t = bcast_row(am_row[:], H, "am")
        bem_t = bcast_row(w_sb["bem"][:], H, "bem")
        hh = cst.tile([B, H], F32, name="hh")
        nc.vector.tensor_tensor(out=hh[:], in0=zc[:], in1=am_t[:B, :],
                                op=mybir.AluOpType.mult)
        nc.vector.tensor_tensor(out=hh[:], in0=hh[:], in1=bem_t[:B, :],
                                op=mybir.AluOpType.add)
        nc.vector.tensor_scalar(out=hh[:], in0=hh[:], scalar1=0.0,
                                scalar2=None, op0=mybir.AluOpType.max)
        hT_ps = ps_tp.tile([H, B], F32, tag="tp", name="hT_ps")
        nc.tensor.transpose(out=hT_ps[:], in_=hh[:], identity=ident[:B, :B])
        hT = cst.tile([H, B], F32, name="hT")
        nc.vector.tensor_copy(out=hT[:], in_=hT_ps[:])
        o_ps = ps_mm.tile([B, OUT], F32, tag="mm", name="o_ps")
        nc.tensor.matmul(out=o_ps[:], lhsT=hT[:], rhs=w_sb["W_m2"][:],
                         start=True, stop=True)
        bm2_t = bcast_row(w_sb["b_m2"][:], OUT, "bm2")
        o_sb = cst.tile([B, OUT], F32, name="o_sb")
        nc.vector.tensor_tensor(out=o_sb[:], in0=o_ps[:], in1=bm2_t[:B, :],
                                op=mybir.AluOpType.add)
        nc.sync.dma_start(out=out_d[:], in_=o_sb[:])

    nc.compile()
    return nc


# --------------------------------------------------------------------------
# entry point
# --------------------------------------------------------------------------

def make_in_maps(inputs, cfg, xt_all, idx_all, bcols_all):
    w = {k: np.ascontiguousarray(np.asarray(v, np.float32))
         for k, v in inputs.items()
         if k not in ("x", "edge_index", "batch")}
    hh = w["att_src"].shape[1]
    shared = {
        "W_gcn": w["W_gcn"], "W_sl": w["W_sl"], "W_sr": w["W_sr"],
        "W_gat": w["W_gat"], "W_m1": w["W_m1"], "W_m2": w["W_m2"],
        "b_gcn": w["b_gcn"].reshape(1, -1), "b_sage": w["b_sage"].reshape(1, -1),
        "b_gat": w["b_gat"].reshape(1, -1),
        "att_s": w["att_src"].reshape(1, -1), "att_d": w["att_dst"].reshape(1, -1),
        "g1": w["g1"].reshape(1, -1), "be1": w["be1"].reshape(1, -1),
        "g2": w["g2"].reshape(1, -1), "be2": w["be2"].reshape(1, -1),
        "g3": w["g3"].reshape(1, -1), "be3": w["be3"].reshape(1, -1),
        "gm": w["gm"].reshape(1, -1), "bem": w["bem"].reshape(1, -1),
        "b_m1": w["b_m1"].reshape(1, -1), "b_m2": w["b_m2"].reshape(1, -1),
    }
    in_maps = []
    for c in range(cfg["ncores"]):
        m = dict(shared)
        m["xT"] = np.ascontiguousarray(xt_all[c])
        m["idx"] = np.ascontiguousarray(idx_all[c])
        m["bcols"] = np.ascontiguousarray(bcols_all[c])
        in_maps.append(m)
    return in_maps


def kernel(**inputs):
    x = np.asarray(inputs["x"], np.float32)
    cfg, xt_all, idx_all, bcols_all = host_prep(
        x, np.asarray(inputs["edge_index"]), np.asarray(inputs["batch"]),
        NCORES)
    nc = build_program(cfg)
    in_maps = make_in_maps(inputs, cfg, xt_all, idx_all, bcols_all)
    res = run_bass_kernel_spmd(nc, in_maps, core_ids=list(range(NCORES)))
    return np.asarray(res.results[0]["out"], np.float32)
